# Initial kernel scaffold
#
"""Paged sparse-attention decode kernel for Trainium2 (8 NeuronCores).

Sharding: one KV head per core (tensor parallel). Each core gathers and
dequantizes its head's paged int8 KV cache, computes the context part of
the attention (unnormalized accumulation + softmax denominator, no
max-subtraction), and the host combines with the current-token term and
performs the (tiny) quantized cache update.
"""
import os
import sys
import numpy as np

for _p in ("/opt/trn_rl_repo", "/root/.axon_site/_ro/trn_rl_repo"):
    if os.path.isdir(_p) and _p not in sys.path:
        sys.path.insert(0, _p)

B = 16
H = 32
HKV = 8
G = H // HKV  # 4
D = 128
BLOCK = 256
MAXBLK = 16
MAXCTX = MAXBLK * BLOCK
NBLK = 256
NSLOTS = NBLK * BLOCK
SCALE = 1.0 / float(np.sqrt(D))
EPS = 1e-8
P = 128
OUTW = G * (D + 4)  # per-batch staging width: 128 accum + denom + pad


def _build_program(block_table, context_lens, nblk, profile=False):
    import concourse.bass as bass
    import concourse.mybir as mybir
    import concourse.tile as tile

    nc = bass.Bass("TRN2", target_bir_lowering=False, debug=False)
    f16 = mybir.dt.float16
    f32 = mybir.dt.float32
    i8 = mybir.dt.int8

    kc = nc.dram_tensor("kc", [NSLOTS, D], i8, kind="ExternalInput")
    vc = nc.dram_tensor("vc", [NSLOTS, D], i8, kind="ExternalInput")
    qT = nc.dram_tensor("qT", [P, B * G], f16, kind="ExternalInput")
    scl = nc.dram_tensor("scl", [P, B * 2 * MAXBLK * 2 * G], f32,
                         kind="ExternalInput")
    ident = nc.dram_tensor("ident", [P, P], f16, kind="ExternalInput")
    out = nc.dram_tensor("out", [G, B * (D + 4)], f32, kind="ExternalOutput")

    with tile.TileContext(nc) as tc:
        with (
            tc.tile_pool(name="cst", bufs=1) as cst,
            tc.tile_pool(name="kv", bufs=3) as kvp,
            tc.tile_pool(name="vres", bufs=2 * MAXBLK + 6) as vres,
            tc.tile_pool(name="kt", bufs=4) as ktp,
            tc.tile_pool(name="soft", bufs=3) as soft,
            tc.tile_pool(name="pp", bufs=8) as pp,
            tc.tile_pool(name="ps_kt", bufs=3, space="PSUM") as ps_kt,
            tc.tile_pool(name="ps_l", bufs=2, space="PSUM") as ps_l,
            tc.tile_pool(name="ps_o", bufs=2, space="PSUM") as ps_o,
        ):
            qt_t = cst.tile([P, B * G], f16)
            nc.sync.dma_start(out=qt_t[:], in_=qT[:])
            scl_t = cst.tile([P, B * 2 * MAXBLK * 2 * G], f32)
            nc.sync.dma_start(out=scl_t[:], in_=scl[:])
            id_t = cst.tile([P, P], f16)
            nc.sync.dma_start(out=id_t[:], in_=ident[:])
            ones_t = cst.tile([P, 1], f16)
            nc.gpsimd.memset(ones_t[:], 1.0)
            stage = cst.tile([G, B * (D + 4)], f32)

            copy_tick = 0
            for b in range(B):
                nb = int(nblk[b])
                if nb == 0:
                    continue
                nch = 2 * nb
                L = int(context_lens[b])

                praw = ps_l.tile([P, P], f32, tag="praw")
                v_tiles = []
                for j in range(nb):
                    blk = int(block_table[b, j])
                    base = blk * BLOCK
                    k_nat = kvp.tile([P, 2, P], f16, tag="k_nat")
                    nc.gpsimd.dma_start(
                        out=k_nat[:].rearrange("p h d -> h p d"),
                        in_=kc[base:base + BLOCK, :].rearrange(
                            "(h p) d -> h p d", p=P),
                    )
                    v_nat = vres.tile([P, 2, P], f16, tag="v_nat")
                    nc.gpsimd.dma_start(
                        out=v_nat[:].rearrange("p h d -> h p d"),
                        in_=vc[base:base + BLOCK, :].rearrange(
                            "(h p) d -> h p d", p=P),
                    )
                    v_tiles.append(v_nat)
                    for c in range(2):
                        gc = 2 * j + c
                        kt_ps = ps_kt.tile([P, P], f16, tag="kt_ps")
                        nc.tensor.transpose(kt_ps[:], k_nat[:, c, :], id_t[:])
                        kt_sb = ktp.tile([P, P], f16, tag="kt_sb")
                        src = kt_ps[:].bitcast(mybir.dt.uint32)
                        dst = kt_sb[:].bitcast(mybir.dt.uint32)
                        if copy_tick % 2 == 0:
                            nc.vector.tensor_copy(dst, src)
                        else:
                            nc.scalar.copy(dst, src)
                        copy_tick += 1
                        nc.tensor.matmul(
                            praw[:, 4 * gc:4 * gc + 4], kt_sb[:],
                            qt_t[:, 4 * b:4 * b + 4],
                            start=True, stop=True,
                        )

                ks_off = (b * 2 + 0) * MAXBLK * 2 * G
                vs_off = (b * 2 + 1) * MAXBLK * 2 * G
                t1 = soft.tile([P, P], f32, tag="t1")
                nc.vector.tensor_mult(
                    t1[:, :4 * nch], praw[:, :4 * nch],
                    scl_t[:, ks_off:ks_off + 4 * nch])
                p_t = pp.tile([P, P], f16, tag="p_t")
                nc.scalar.activation(
                    p_t[:, :4 * nch], t1[:, :4 * nch],
                    mybir.ActivationFunctionType.Exp)
                # mask invalid tail slots: s in [L, 256*nb)
                gc0 = L // P
                r0 = L % P
                if r0 > 0 and gc0 < nch:
                    nc.any.memset(p_t[r0:P, 4 * gc0:4 * gc0 + 4], 0.0)
                gc1 = (L + P - 1) // P
                if gc1 < nch:
                    nc.any.memset(p_t[:, 4 * gc1:4 * nch], 0.0)
                p2 = pp.tile([P, P], f16, tag="p2")
                nc.vector.tensor_mult(
                    p2[:, :4 * nch], p_t[:, :4 * nch],
                    scl_t[:, vs_off:vs_off + 4 * nch])

                o_ps = ps_o.tile([G, D + 4], f32, tag="o_ps")
                for j in range(nb):
                    v_nat = v_tiles[j]
                    for c in range(2):
                        gc = 2 * j + c
                        nc.tensor.matmul(
                            o_ps[:, :D], p2[:, 4 * gc:4 * gc + 4],
                            v_nat[:, c, :],
                            start=(gc == 0), stop=(gc == nch - 1),
                        )
                        nc.tensor.matmul(
                            o_ps[:, D:D + 1], p_t[:, 4 * gc:4 * gc + 4],
                            ones_t[:],
                            start=(gc == 0), stop=(gc == nch - 1),
                        )
                nc.vector.tensor_copy(
                    stage[:, b * (D + 4):(b + 1) * (D + 4)], o_ps[:])

            nc.sync.dma_start(out=out[:], in_=stage[:])
    return nc


def kernel(q, k, v, k_cache, v_cache, k_scale_cache, v_scale_cache,
           slot_mapping, block_table, context_lens):
    from concourse.bass_utils import run_bass_kernel_spmd

    q = np.asarray(q)
    k = np.asarray(k)
    v = np.asarray(v)
    k_cache_in = np.asarray(k_cache)
    v_cache_in = np.asarray(v_cache)
    k_scale_cache = np.asarray(k_scale_cache, dtype=np.float32)
    v_scale_cache = np.asarray(v_scale_cache, dtype=np.float32)
    slot_mapping = np.asarray(slot_mapping).astype(np.int64)
    block_table = np.asarray(block_table).astype(np.int64)
    context_lens = np.asarray(context_lens).astype(np.int64)

    kc8 = k_cache_in.astype(np.int8) if k_cache_in.dtype != np.int8 else k_cache_in
    vc8 = v_cache_in.astype(np.int8) if v_cache_in.dtype != np.int8 else v_cache_in

    nblk = np.minimum((context_lens + BLOCK - 1) // BLOCK, MAXBLK)

    nc = _build_program(block_table, context_lens, nblk)

    # host-side slot list per batch in chunk-column layout [128, nch]
    pos = np.arange(MAXCTX)
    slots = block_table[:, pos // BLOCK] * BLOCK + (pos % BLOCK)  # [B, S]

    qf = np.asarray(q, dtype=np.float32)
    in_maps = []
    for h in range(HKV):
        qh = qf[:, h * G:(h + 1) * G, :]  # [B, G, D]
        qT = np.ascontiguousarray(
            qh.reshape(B * G, D).T.astype(np.float16))  # [128, 64]
        scl = np.zeros((P, B * 2 * MAXBLK * 2 * G), dtype=np.float32)
        for b in range(B):
            nch = 2 * int(nblk[b])
            if nch == 0:
                continue
            sl = slots[b, :nch * P].reshape(nch, P)  # [nch, 128]
            ks = k_scale_cache[h, sl].T * SCALE      # [128, nch]
            vs = v_scale_cache[h, sl].T              # [128, nch]
            ks4 = np.repeat(ks, G, axis=1)           # [128, nch*4]
            vs4 = np.repeat(vs, G, axis=1)
            ko = (b * 2 + 0) * MAXBLK * 2 * G
            vo = (b * 2 + 1) * MAXBLK * 2 * G
            scl[:, ko:ko + 4 * nch] = ks4
            scl[:, vo:vo + 4 * nch] = vs4
        in_maps.append(dict(
            kc=kc8[h], vc=vc8[h], qT=qT, scl=scl,
            ident=np.eye(P, dtype=np.float16),
        ))

    res = run_bass_kernel_spmd(nc, in_maps, list(range(HKV)))

    # ---- host combine: current-token term + normalization ----
    o = np.zeros((B, H, D), dtype=np.float32)
    for h in range(HKV):
        dev = res.results[h]["out"].reshape(G, B, D + 4)  # [G, B, 132]
        qh = qf[:, h * G:(h + 1) * G, :]                  # [B, G, D]
        lcur = np.einsum("bgd,bd->bg", qh, k[:, h, :].astype(np.float32))
        pcur = np.exp(lcur * np.float32(SCALE))           # [B, G]
        for b in range(B):
            if nblk[b] == 0:
                acc = np.zeros((G, D), dtype=np.float32)
                den = np.zeros((G,), dtype=np.float32)
            else:
                acc = dev[:, b, :D]
                den = dev[:, b, D]
            num = acc + pcur[b][:, None] * v[b, h, :].astype(np.float32)[None, :]
            o[b, h * G:(h + 1) * G, :] = num / (den + pcur[b])[:, None]

    # ---- quantized KV-cache store (host, exact reference semantics) ----
    def _quantize(x):
        x = np.asarray(x, dtype=np.float32)
        s = np.maximum(np.max(np.abs(x), axis=-1) / np.float32(127.0),
                       np.float32(EPS))
        xi = np.clip(np.round(x / s[..., None]), -127.0, 127.0).astype(np.int8)
        return xi, s.astype(np.float32)

    kq8, ks = _quantize(k)  # [B, HKV, D], [B, HKV]
    vq8, vs = _quantize(v)
    kc_out = k_cache_in.copy()
    vc_out = v_cache_in.copy()
    ksc = k_scale_cache.copy()
    vsc = v_scale_cache.copy()
    kc_out[:, slot_mapping, :] = np.transpose(kq8, (1, 0, 2))
    vc_out[:, slot_mapping, :] = np.transpose(vq8, (1, 0, 2))
    ksc[:, slot_mapping] = ks.T
    vsc[:, slot_mapping] = vs.T

    return (o, kc_out, vc_out, ksc, vsc)


# revision 17
# speedup vs baseline: 1.0096x; 1.0096x over previous
"""Paged sparse-attention decode kernel for Trainium2 (8 NeuronCores).

Sharding: one KV head per core (tensor parallel). Each core gathers and
dequantizes its head's paged int8 KV cache, computes the context part of
the attention (unnormalized accumulation + softmax denominator, no
max-subtraction), and the host combines with the current-token term and
performs the (tiny) quantized cache update.
"""
import os
import sys
import numpy as np

for _p in ("/opt/trn_rl_repo", "/root/.axon_site/_ro/trn_rl_repo"):
    if os.path.isdir(_p) and _p not in sys.path:
        sys.path.insert(0, _p)

B = 16
H = 32
HKV = 8
G = H // HKV  # 4
D = 128
BLOCK = 256
MAXBLK = 16
MAXCTX = MAXBLK * BLOCK
NBLK = 256
NSLOTS = NBLK * BLOCK
SCALE = 1.0 / float(np.sqrt(D))
EPS = 1e-8
P = 128
LAST_RESULT = None
OUTW = G * (D + 4)  # per-batch staging width: 128 accum + denom + pad


def _build_program(block_table, context_lens, nblk, profile=False):
    import concourse.mybir as mybir
    import concourse.tile as tile
    from concourse import bacc

    ABL = set(os.environ.get("BASSK_ABLATE", "").split(","))

    nc = bacc.Bacc("TRN2", target_bir_lowering=False, debug=False)
    f16 = mybir.dt.float16
    f32 = mybir.dt.float32
    i8 = mybir.dt.int8

    kc = nc.dram_tensor("kc", [NSLOTS, D], i8, kind="ExternalInput")
    vc = nc.dram_tensor("vc", [NSLOTS, D], i8, kind="ExternalInput")
    qT = nc.dram_tensor("qT", [P, B * G], f16, kind="ExternalInput")
    scl = nc.dram_tensor("scl", [P, B * 3 * MAXBLK * 2 * G], f32,
                         kind="ExternalInput")
    ident = nc.dram_tensor("ident", [P, P], f16, kind="ExternalInput")
    out = nc.dram_tensor("out", [G, B * (D + 4)], f32, kind="ExternalOutput")

    with tile.TileContext(nc) as tc:
        with (
            tc.tile_pool(name="cst", bufs=1) as cst,
            tc.tile_pool(name="st8", bufs=3) as st8,
            tc.tile_pool(name="f16", bufs=3) as f16p,
            tc.tile_pool(name="kt", bufs=4) as ktp,
            tc.tile_pool(name="soft", bufs=3) as soft,
            tc.tile_pool(name="pp", bufs=8) as pp,
            tc.tile_pool(name="ps_kt", bufs=3, space="PSUM") as ps_kt,
            tc.tile_pool(name="ps_l", bufs=2, space="PSUM") as ps_l,
            tc.tile_pool(name="ps_o", bufs=2, space="PSUM") as ps_o,
        ):
            qt_t = cst.tile([P, B * G], f16)
            nc.sync.dma_start(out=qt_t[:], in_=qT[:])
            scl_t = cst.tile([P, B * 3 * MAXBLK * 2 * G], f32)
            nc.sync.dma_start(out=scl_t[:], in_=scl[:])
            id_t = cst.tile([P, P], f16)
            nc.sync.dma_start(out=id_t[:], in_=ident[:])
            ones_t = cst.tile([P, 1], f16)
            nc.gpsimd.memset(ones_t[:], 1.0)
            stage = cst.tile([G, B * (D + 4)], f32)
            nc.gpsimd.memset(stage[:], 0.0)

            copy_tick = 0
            for b in range(B):
                nb = int(nblk[b])
                if nb == 0:
                    continue
                nch = 2 * nb
                L = int(context_lens[b])

                praw = ps_l.tile([P, P], f32, tag="praw")
                kst = st8.tile([P, 2 * MAXBLK, P], i8, tag="kst")
                vst = st8.tile([P, 2 * MAXBLK, P], i8, tag="vst")
                for j in range(nb):
                    if "noload" in ABL:
                        break
                    blk = int(block_table[b, j])
                    base = blk * BLOCK
                    for c in range(2):
                        gc = 2 * j + c
                        nc.sync.dma_start(
                            out=kst[:, gc, :],
                            in_=kc[base + c * P:base + (c + 1) * P, :])
                        nc.scalar.dma_start(
                            out=vst[:, gc, :],
                            in_=vc[base + c * P:base + (c + 1) * P, :])
                kf = f16p.tile([P, 2 * MAXBLK, P], f16, tag="kf")
                vf = f16p.tile([P, 2 * MAXBLK, P], f16, tag="vf")
                if "nocast" not in ABL:
                    nc.gpsimd.dma_start(out=kf[:, :nch, :], in_=kst[:, :nch, :])
                    nc.gpsimd.dma_start(out=vf[:, :nch, :], in_=vst[:, :nch, :])
                for gc in range(nch if "noqk" not in ABL else 0):
                    kt_ps = ps_kt.tile([P, P], f16, tag="kt_ps")
                    nc.tensor.transpose(kt_ps[:], kf[:, gc, :], id_t[:])
                    kt_sb = ktp.tile([P, P], f16, tag="kt_sb")
                    if copy_tick % 2 == 0:
                        nc.vector.tensor_copy(kt_sb[:], kt_ps[:])
                    else:
                        nc.scalar.copy(kt_sb[:], kt_ps[:])
                    copy_tick += 1
                    nc.tensor.matmul(
                        praw[:, 4 * gc:4 * gc + 4], kt_sb[:],
                        qt_t[:, 4 * b:4 * b + 4],
                        start=True, stop=True,
                    )

                ks_off = (b * 3 + 0) * MAXBLK * 2 * G
                ms_off = (b * 3 + 1) * MAXBLK * 2 * G
                vs_off = (b * 3 + 2) * MAXBLK * 2 * G
                if "nosoftmax" in ABL:
                    continue
                t1 = soft.tile([P, P], f32, tag="t1")
                nc.vector.tensor_mul(
                    t1[:, :4 * nch], praw[:, :4 * nch],
                    scl_t[:, ks_off:ks_off + 4 * nch])
                t2 = soft.tile([P, P], f32, tag="t2")
                nc.vector.tensor_add(
                    t2[:, :4 * nch], t1[:, :4 * nch],
                    scl_t[:, ms_off:ms_off + 4 * nch])
                p_t = pp.tile([P, P], f16, tag="p_t")
                nc.scalar.activation(
                    p_t[:, :4 * nch], t2[:, :4 * nch],
                    mybir.ActivationFunctionType.Exp)
                p2 = pp.tile([P, P], f16, tag="p2")
                nc.vector.tensor_mul(
                    p2[:, :4 * nch], p_t[:, :4 * nch],
                    scl_t[:, vs_off:vs_off + 4 * nch])

                o_ps = ps_o.tile([G, D + 4], f32, tag="o_ps")
                for gc in range(nch if "nopv" not in ABL else 0):
                    nc.tensor.matmul(
                        o_ps[:, :D], p2[:, 4 * gc:4 * gc + 4],
                        vf[:, gc, :],
                        start=(gc == 0), stop=(gc == nch - 1),
                    )
                for gc in range(nch if "noden" not in ABL else 0):
                    nc.tensor.matmul(
                        o_ps[:, D:D + 1], p_t[:, 4 * gc:4 * gc + 4],
                        ones_t[:],
                        start=(gc == 0), stop=(gc == nch - 1),
                    )
                nc.vector.tensor_copy(
                    stage[:, b * (D + 4):b * (D + 4) + D + 1],
                    o_ps[:, :D + 1])

            nc.sync.dma_start(out=out[:], in_=stage[:])
    nc.finalize()
    return nc


def kernel(q, k, v, k_cache, v_cache, k_scale_cache, v_scale_cache,
           slot_mapping, block_table, context_lens):
    from concourse.bass_utils import run_bass_kernel_spmd

    q = np.asarray(q)
    k = np.asarray(k)
    v = np.asarray(v)
    k_cache_in = np.asarray(k_cache)
    v_cache_in = np.asarray(v_cache)
    k_scale_cache = np.asarray(k_scale_cache, dtype=np.float32)
    v_scale_cache = np.asarray(v_scale_cache, dtype=np.float32)
    slot_mapping = np.asarray(slot_mapping).astype(np.int64)
    block_table = np.asarray(block_table).astype(np.int64)
    context_lens = np.asarray(context_lens).astype(np.int64)

    kc8 = k_cache_in.astype(np.int8) if k_cache_in.dtype != np.int8 else k_cache_in
    vc8 = v_cache_in.astype(np.int8) if v_cache_in.dtype != np.int8 else v_cache_in

    nblk = np.minimum((context_lens + BLOCK - 1) // BLOCK, MAXBLK)

    nc = _build_program(block_table, context_lens, nblk)

    # host-side slot list per batch in chunk-column layout [128, nch]
    pos = np.arange(MAXCTX)
    slots = block_table[:, pos // BLOCK] * BLOCK + (pos % BLOCK)  # [B, S]

    qf = np.asarray(q, dtype=np.float32)
    in_maps = []
    for h in range(HKV):
        qh = qf[:, h * G:(h + 1) * G, :]  # [B, G, D]
        qT = np.ascontiguousarray(
            qh.reshape(B * G, D).T.astype(np.float16))  # [128, 64]
        scl = np.zeros((P, B * 3 * MAXBLK * 2 * G), dtype=np.float32)
        for b in range(B):
            nch = 2 * int(nblk[b])
            if nch == 0:
                continue
            L = int(context_lens[b])
            sl = slots[b, :nch * P].reshape(nch, P)  # [nch, 128]
            ks = k_scale_cache[h, sl].T * SCALE      # [128, nch]
            vs = v_scale_cache[h, sl].T              # [128, nch]
            valid = (np.arange(nch * P).reshape(nch, P).T < L)  # [128, nch]
            msk = np.where(valid, np.float32(0.0), np.float32(-1e30))
            ks4 = np.repeat(ks, G, axis=1)           # [128, nch*4]
            vs4 = np.repeat(vs, G, axis=1)
            ms4 = np.repeat(msk, G, axis=1)
            ko = (b * 3 + 0) * MAXBLK * 2 * G
            mo = (b * 3 + 1) * MAXBLK * 2 * G
            vo = (b * 3 + 2) * MAXBLK * 2 * G
            scl[:, ko:ko + 4 * nch] = ks4
            scl[:, mo:mo + 4 * nch] = ms4
            scl[:, vo:vo + 4 * nch] = vs4
        in_maps.append(dict(
            kc=kc8[h], vc=vc8[h], qT=qT, scl=scl,
            ident=np.eye(P, dtype=np.float16),
        ))

    trace = bool(os.environ.get("BASSK_PROFILE"))
    res = run_bass_kernel_spmd(nc, in_maps, list(range(HKV)), trace=trace)
    if trace:
        global LAST_RESULT
        LAST_RESULT = res

    # ---- host combine: current-token term + normalization ----
    o = np.zeros((B, H, D), dtype=np.float32)
    for h in range(HKV):
        dev = res.results[h]["out"].reshape(G, B, D + 4)  # [G, B, 132]
        qh = qf[:, h * G:(h + 1) * G, :]                  # [B, G, D]
        lcur = np.einsum("bgd,bd->bg", qh, k[:, h, :].astype(np.float32))
        pcur = np.exp(lcur * np.float32(SCALE))           # [B, G]
        for b in range(B):
            if nblk[b] == 0:
                acc = np.zeros((G, D), dtype=np.float32)
                den = np.zeros((G,), dtype=np.float32)
            else:
                acc = dev[:, b, :D]
                den = dev[:, b, D]
            num = acc + pcur[b][:, None] * v[b, h, :].astype(np.float32)[None, :]
            o[b, h * G:(h + 1) * G, :] = num / (den + pcur[b])[:, None]

    # ---- quantized KV-cache store (host, exact reference semantics) ----
    def _quantize(x):
        x = np.asarray(x, dtype=np.float32)
        s = np.maximum(np.max(np.abs(x), axis=-1) / np.float32(127.0),
                       np.float32(EPS))
        xi = np.clip(np.round(x / s[..., None]), -127.0, 127.0).astype(np.int8)
        return xi, s.astype(np.float32)

    kq8, ks = _quantize(k)  # [B, HKV, D], [B, HKV]
    vq8, vs = _quantize(v)
    kc_out = k_cache_in.copy()
    vc_out = v_cache_in.copy()
    ksc = k_scale_cache.copy()
    vsc = v_scale_cache.copy()
    kc_out[:, slot_mapping, :] = np.transpose(kq8, (1, 0, 2))
    vc_out[:, slot_mapping, :] = np.transpose(vq8, (1, 0, 2))
    ksc[:, slot_mapping] = ks.T
    vsc[:, slot_mapping] = vs.T

    return (o, kc_out, vc_out, ksc, vsc)
